# revision 24
# baseline (speedup 1.0000x reference)
"""Trainium2 Bass kernel for nn_LoraBigNet (18x LoRALinear MLP, 6 residual
blocks with inter-block LayerNorm).

Strategy: data-parallel over the batch dim (16384 rows -> 2048 rows/core on 8
cores), parameters replicated. The frozen LoRA low-rank path is folded into
the main weights on the host (W'' = fp16(W + Bm @ A)) — exact same function,
~25% fewer PE cycles on chip. Activations live on-chip in transposed layout
h.T [din(partitions) x n(free)] so every matmul contracts din on the partition
axis with zero on-chip transposes; weights are pre-transposed on the host.
PSUM evacuation applies bias (+ ReLU) on the Scalar engine, rounding to fp16
exactly like the fp16 reference path. LayerNorm reduces over the partition
axis with ones-vector matmuls (for blocks past the first, the sum comes free
from the fp16 residual-branch tiles since the previous LN output has zero
column-sums) and broadcasts mean/rstd back over partitions with K=1 matmuls.
"""

import numpy as np

import concourse.bass as bass
import concourse.mybir as mybir
from concourse.tile import TileContext
from concourse.bass_utils import run_bass_kernel_spmd

F16 = mybir.dt.float16
F32 = mybir.dt.float32
AF = mybir.ActivationFunctionType
OP = mybir.AluOpType

N, D, R, NLIN = 16384, 1024, 32, 18
CORES = 8
NS = N // CORES          # rows per core
KT = D // 128            # contraction tiles
DT = D // 128            # output tiles
CH = 512                 # matmul moving free-dim chunk
P = 128


def _split_waits(nc, maxw=1):
    """This walrus build rejects more than one sync-wait per instruction.
    Hoist extra waits onto preceding same-engine nops — the issuing sequencer
    executes them in order, so the semantics are identical."""
    ctr = 0
    for f in nc.m.functions:
        for bb in f.blocks:
            insts = list(bb.instructions)
            out = []
            changed = False
            for inst in insts:
                si = getattr(inst, "sync_info", None)
                waits = list(si.on_wait) if si and si.on_wait else []
                if len(waits) > maxw:
                    changed = True
                    for w in waits[:-maxw]:
                        nop = mybir.InstNoOp(
                            name=f"wsplit_{ctr}", ins=[], outs=[],
                            engine=inst.engine,
                        )
                        ctr += 1
                        nop.sync_info = mybir.SyncInfo(on_wait=[w], on_update=[])
                        nc.register_instruction(nop, overwrite=True)
                        out.append(nop)
                    inst.sync_info = mybir.SyncInfo(
                        on_wait=waits[-maxw:], on_update=list(si.on_update)
                    )
                out.append(inst)
            if changed:
                bb.instructions = out


def build(ns=NS, n_blocks=6, ln_b_nonzero=False, ln_trivial=True):
    """Build the single-core SPMD Bass program."""
    assert ns % CH == 0
    nlin = 3 * n_blocks
    nln = max(n_blocks - 1, 1)
    nch = ns // CH

    nc = bass.Bass()
    xT = nc.declare_dram_parameter("xT", [D, ns], F32, False)
    WTd = nc.declare_dram_parameter("WT", [nlin, D, D], F16, False)
    BRd = nc.declare_dram_parameter("BR", [P, nlin * DT], F16, False)
    GRd = nc.declare_dram_parameter("GR", [P, nln * KT], F32, False)
    LBd = None
    if ln_b_nonzero:
        LBd = nc.declare_dram_parameter("LB", [P, nln * KT], F32, False)
    outT = nc.declare_dram_parameter("outT", [D, ns], F32, True)

    with TileContext(nc) as tc:
        with (
            tc.tile_pool(name="const", bufs=1) as const,
            tc.tile_pool(name="h32p", bufs=1) as h32p,
            tc.tile_pool(name="h16p", bufs=2) as h16p,
            tc.tile_pool(name="wtp", bufs=2) as wtp,
            tc.tile_pool(name="y3p", bufs=4) as y3p,
            tc.tile_pool(name="lnsc", bufs=2) as lnsc,
            tc.tile_pool(name="rowp", bufs=1) as rowp,
            tc.tile_pool(name="cenp", bufs=2) as cenp,
            tc.tile_pool(name="mup", bufs=4) as mup,
            tc.tile_pool(name="drp", bufs=2, space="DRAM") as drp,
            tc.tile_pool(name="bcp", bufs=2) as bcp,
            tc.tile_pool(name="psmm", bufs=6, space="PSUM") as pmp,
            tc.tile_pool(name="psst", bufs=1, space="PSUM") as pst,
        ):
            ones16 = const.tile([P, 1], F16)
            nc.vector.memset(ones16, 1.0)
            onesr = const.tile([1, P], F32)
            nc.vector.memset(onesr, 1.0)
            epsT = const.tile([1, 1], F32)
            nc.vector.memset(epsT, 1e-5)
            ball = const.tile([P, nlin * DT], F16)
            nc.sync.dma_start(out=ball, in_=BRd[:])
            gall = const.tile([P, nln * KT], F32)
            nc.sync.dma_start(out=gall, in_=GRd[:])
            lball = None
            if LBd is not None:
                lball = const.tile([P, nln * KT], F32)
                nc.sync.dma_start(out=lball, in_=LBd[:])

            wt0 = wtp.tile([P, KT, D], F16, name="wt0", tag="wt")
            nc.sync.dma_start(out=wt0[:], in_=WTd[0].rearrange("(k p) d -> p k d", p=P))

            H32 = h32p.tile([P, KT, ns], F32)
            xTr = xT.rearrange("(k p) n -> p k n", p=P)
            cur16 = h16p.tile([P, KT, ns], F16, tag="h16")
            for c in range(nch):
                sl = slice(c * CH, (c + 1) * CH)
                for k in range(KT):
                    nc.sync.dma_start(out=H32[:, k, sl], in_=xTr[:, k, sl])
                for k in range(KT):
                    nc.scalar.copy(out=cur16[:, k, sl], in_=H32[:, k, sl])

            for blk in range(n_blocks):
                has_ln = blk < n_blocks - 1
                # S1 from y3 tiles directly (sum of the carried h0 is ~0 when
                # it is a previous LayerNorm output with trivial gain/bias)
                s1_from_y3 = has_ln and blk > 0 and ln_trivial
                mus = [None] * nch
                mss = [None] * nch
                for li in range(3):
                    i = 3 * blk + li
                    if i == 0:
                        wt = wt0
                    else:
                        wt = wtp.tile([P, KT, D], F16, tag="wt")
                        nc.sync.dma_start(
                            out=wt[:], in_=WTd[i].rearrange("(k p) d -> p k d", p=P)
                        )
                    dst16 = None
                    if li < 2:
                        dst16 = h16p.tile([P, KT, ns], F16, name=f"dst16_{i}", tag="h16")
                    for c in range(nch):
                        sl = slice(c * CH, (c + 1) * CH)
                        s1p = s2p = None
                        if li == 2 and has_ln:
                            s1p = pst.tile([1, CH], F32, name=f"s1_{blk}_{c}", tag="s1")
                            s2p = pst.tile([1, CH], F32, name=f"s2_{blk}_{c}", tag="s2")
                        for d in range(DT):
                            mp = pmp.tile([P, CH], F32, tag="m")
                            for k in range(KT):
                                nc.tensor.matmul(
                                    mp,
                                    lhsT=wt[:, k, d * P : (d + 1) * P],
                                    rhs=cur16[:, k, sl],
                                    start=(k == 0),
                                    stop=(k == KT - 1),
                                )
                            bap = ball[:, i * DT + d : i * DT + d + 1]
                            if li < 2:
                                nc.scalar.activation(
                                    out=dst16[:, d, sl],
                                    in_=mp,
                                    func=AF.Relu,
                                    bias=bap,
                                    scale=1.0,
                                )
                            else:
                                y3 = y3p.tile([P, CH], F16, tag="y3")
                                nc.scalar.activation(
                                    out=y3, in_=mp, func=AF.Identity,
                                    bias=bap, scale=1.0,
                                )
                                nc.vector.tensor_add(
                                    H32[:, d, sl], H32[:, d, sl], y3
                                )
                                if has_ln:
                                    if s1_from_y3:
                                        nc.tensor.matmul(
                                            s1p, lhsT=ones16, rhs=y3,
                                            start=(d == 0), stop=(d == DT - 1),
                                        )
                                    else:
                                        r16 = lnsc.tile([P, CH], F16, tag="r16")
                                        nc.vector.tensor_copy(r16, H32[:, d, sl])
                                        nc.tensor.matmul(
                                            s1p, lhsT=ones16, rhs=r16,
                                            start=(d == 0), stop=(d == DT - 1),
                                        )
                                    rsq = lnsc.tile([P, CH], F16, tag="rsq")
                                    nc.scalar.activation(
                                        out=rsq, in_=H32[:, d, sl], func=AF.Square
                                    )
                                    nc.tensor.matmul(
                                        s2p, lhsT=ones16, rhs=rsq,
                                        start=(d == 0), stop=(d == DT - 1),
                                    )
                        if li == 2 and has_ln:
                            mu_t = mup.tile([1, CH], F32, tag="mu")
                            nc.scalar.mul(out=mu_t, in_=s1p, mul=1.0 / D)
                            ms_t = mup.tile([1, CH], F32, tag="ms")
                            nc.scalar.mul(out=ms_t, in_=s2p, mul=1.0 / D)
                            mus[c] = mu_t
                            mss[c] = ms_t
                    if li < 2:
                        cur16 = dst16

                if has_ln:
                    # ---- LayerNorm over din (partition axis) ----
                    j = blk
                    new16 = h16p.tile([P, KT, ns], F16, name=f"ln16_{j}", tag="h16")
                    for c in range(nch):
                        sl = slice(c * CH, (c + 1) * CH)
                        mu = mus[c]
                        var = rowp.tile([1, CH], F32, tag="var")
                        nc.vector.tensor_mul(var, mu, mu)
                        nc.vector.tensor_sub(var, mss[c], var)
                        # rsqrt via ACT tables: s = exp(-0.5 * ln(var + eps))
                        sd = rowp.tile([1, CH], F32, tag="sd")
                        nc.scalar.activation(
                            out=sd, in_=var, func=AF.Ln, bias=epsT, scale=1.0
                        )
                        srow = rowp.tile([1, CH], F32, tag="srow")
                        nc.scalar.activation(
                            out=srow, in_=sd, func=AF.Exp, bias=0.0, scale=-0.5
                        )
                        dmu = drp.tile([1, CH], F32, tag="dmu")
                        nc.sync.dma_start(out=dmu, in_=mu)
                        dsr = drp.tile([1, CH], F32, tag="dsr")
                        nc.sync.dma_start(out=dsr, in_=srow)
                        bmu = bcp.tile([P, CH], F32, tag="bmu")
                        nc.sync.dma_start(out=bmu, in_=dmu.to_broadcast([P, CH]))
                        bsc = bcp.tile([P, CH], F32, tag="bs")
                        nc.sync.dma_start(out=bsc, in_=dsr.to_broadcast([P, CH]))
                        for k in range(KT):
                            cen = cenp.tile([P, CH], F32, tag="cen")
                            nc.vector.tensor_sub(cen, H32[:, k, sl], bmu)
                            gap = gall[:, j * KT + k : j * KT + k + 1]
                            nc.vector.scalar_tensor_tensor(
                                out=H32[:, k, sl],
                                in0=cen,
                                scalar=gap,
                                in1=bsc,
                                op0=OP.mult,
                                op1=OP.mult,
                            )
                            if lball is not None:
                                lbap = lball[:, j * KT + k : j * KT + k + 1]
                                nc.vector.tensor_scalar_add(
                                    H32[:, k, sl], H32[:, k, sl], lbap
                                )
                            nc.scalar.copy(out=new16[:, k, sl], in_=H32[:, k, sl])
                    cur16 = new16
                else:
                    for k in range(KT):
                        nc.sync.dma_start(
                            out=outT[k * P : (k + 1) * P, :], in_=H32[:, k, :]
                        )
    _split_waits(nc)
    return nc


def _prep_params(W, b, A, Bm, ln_g, ln_b, n_blocks=6):
    nlin = 3 * n_blocks
    nln = max(n_blocks - 1, 1)
    # fold the frozen LoRA path into the main weights: W'' = fp16(W + Bm @ A)
    Wf = (
        W[:nlin].astype(np.float32)
        + np.matmul(Bm[:nlin].astype(np.float32), A[:nlin].astype(np.float32))
    ).astype(np.float16)
    WT = np.ascontiguousarray(Wf.transpose(0, 2, 1))           # [nlin, din, dout] f16
    BR = np.ascontiguousarray(
        b[:nlin].reshape(nlin, DT, P).transpose(2, 0, 1).reshape(P, nlin * DT)
    )
    g = ln_g[:nln] if ln_g.shape[0] >= nln else np.ones((nln, D), np.float32)
    GR = np.ascontiguousarray(
        g.reshape(nln, KT, P).transpose(2, 0, 1).reshape(P, nln * KT)
    ).astype(np.float32)
    lb = ln_b[:nln] if ln_b.shape[0] >= nln else np.zeros((nln, D), np.float32)
    ln_b_nonzero = bool(np.any(lb != 0))
    LB = np.ascontiguousarray(
        lb.reshape(nln, KT, P).transpose(2, 0, 1).reshape(P, nln * KT)
    ).astype(np.float32)
    ln_trivial = bool(np.all(g == 1.0)) and not ln_b_nonzero
    return WT, BR, GR, LB, ln_b_nonzero, ln_trivial


_nc_cache = {}


def run(x, W, b, A, Bm, ln_g, ln_b, n_blocks=6, trace=False, tmpdir=None):
    ns = x.shape[0] // CORES
    WT, BR, GR, LB, ln_b_nonzero, ln_trivial = _prep_params(W, b, A, Bm, ln_g, ln_b, n_blocks)
    key = (ns, n_blocks, ln_b_nonzero, ln_trivial)
    if key not in _nc_cache:
        _nc_cache[key] = build(ns, n_blocks, ln_b_nonzero, ln_trivial)
    nc = _nc_cache[key]

    in_maps = []
    for c in range(CORES):
        m = {
            "xT": np.ascontiguousarray(x[c * ns : (c + 1) * ns, :].T),
            "WT": WT, "BR": BR, "GR": GR,
        }
        if ln_b_nonzero:
            m["LB"] = LB
        in_maps.append(m)

    res = run_bass_kernel_spmd(
        nc, in_maps, list(range(CORES)), trace=trace, tmpdir=tmpdir
    )
    out = np.empty((x.shape[0], D), np.float32)
    for c in range(CORES):
        out[c * ns : (c + 1) * ns, :] = res.results[c]["outT"].T
    return out, res


def kernel(x, W, b, A, Bm, ln_g, ln_b):
    out, _ = run(
        np.asarray(x), np.asarray(W), np.asarray(b), np.asarray(A),
        np.asarray(Bm), np.asarray(ln_g), np.asarray(ln_b),
    )
    return out


# revision 25
# speedup vs baseline: 1.1226x; 1.1226x over previous
"""Trainium2 Bass kernel for nn_LoraBigNet (18x LoRALinear MLP, 6 residual
blocks with inter-block LayerNorm).

Strategy: data-parallel over the batch dim (16384 rows -> 2048 rows/core on 8
cores), parameters replicated. The frozen LoRA low-rank path is folded into
the main weights on the host (W'' = fp16(W + Bm @ A)) — exact same function,
~25% fewer PE cycles on chip. Activations live on-chip in transposed layout
h.T [din(partitions) x n(free)] so every matmul contracts din on the partition
axis with zero on-chip transposes; weights are pre-transposed on the host.
PSUM evacuation applies bias (+ ReLU) on the Scalar engine, rounding to fp16
exactly like the fp16 reference path. LayerNorm reduces over the partition
axis with ones-vector matmuls (for blocks past the first, the sum comes free
from the fp16 residual-branch tiles since the previous LN output has zero
column-sums) and broadcasts mean/rstd back over partitions with K=1 matmuls.
"""

import numpy as np

import concourse.bass as bass
import concourse.mybir as mybir
from concourse.tile import TileContext
from concourse.bass_utils import run_bass_kernel_spmd

F16 = mybir.dt.float16
F32 = mybir.dt.float32
AF = mybir.ActivationFunctionType
OP = mybir.AluOpType

N, D, R, NLIN = 16384, 1024, 32, 18
CORES = 8
NS = N // CORES          # rows per core
KT = D // 128            # contraction tiles
DT = D // 128            # output tiles
CH = 512                 # matmul moving free-dim chunk
P = 128


def _split_waits(nc, maxw=1):
    """This walrus build rejects more than one sync-wait per instruction.
    Hoist extra waits onto preceding same-engine nops — the issuing sequencer
    executes them in order, so the semantics are identical."""
    ctr = 0
    for f in nc.m.functions:
        for bb in f.blocks:
            insts = list(bb.instructions)
            out = []
            changed = False
            for inst in insts:
                si = getattr(inst, "sync_info", None)
                waits = list(si.on_wait) if si and si.on_wait else []
                if len(waits) > maxw:
                    changed = True
                    for w in waits[:-maxw]:
                        nop = mybir.InstNoOp(
                            name=f"wsplit_{ctr}", ins=[], outs=[],
                            engine=inst.engine,
                        )
                        ctr += 1
                        nop.sync_info = mybir.SyncInfo(on_wait=[w], on_update=[])
                        nc.register_instruction(nop, overwrite=True)
                        out.append(nop)
                    inst.sync_info = mybir.SyncInfo(
                        on_wait=waits[-maxw:], on_update=list(si.on_update)
                    )
                out.append(inst)
            if changed:
                bb.instructions = out


def build(ns=NS, n_blocks=6, ln_b_nonzero=False, ln_trivial=True):
    """Build the single-core SPMD Bass program."""
    assert ns % CH == 0
    nlin = 3 * n_blocks
    nln = max(n_blocks - 1, 1)
    nch = ns // CH

    nc = bass.Bass()
    xT = nc.declare_dram_parameter("xT", [D, ns], F32, False)
    WTd = nc.declare_dram_parameter("WT", [nlin, D, D], F16, False)
    BRd = nc.declare_dram_parameter("BR", [P, nlin * DT], F16, False)
    GRd = nc.declare_dram_parameter("GR", [P, nln * KT], F32, False)
    LBd = None
    if ln_b_nonzero:
        LBd = nc.declare_dram_parameter("LB", [P, nln * KT], F32, False)
    outT = nc.declare_dram_parameter("outT", [D, ns], F32, True)

    with TileContext(nc) as tc:
        with (
            tc.tile_pool(name="const", bufs=1) as const,
            tc.tile_pool(name="h32p", bufs=1) as h32p,
            tc.tile_pool(name="h16p", bufs=2) as h16p,
            tc.tile_pool(name="wtp", bufs=2) as wtp,
            tc.tile_pool(name="y3p", bufs=4) as y3p,
            tc.tile_pool(name="lnsc", bufs=2) as lnsc,
            tc.tile_pool(name="rowp", bufs=2) as rowp,
            tc.tile_pool(name="cenp", bufs=2) as cenp,
            tc.tile_pool(name="mup", bufs=4) as mup,
            tc.tile_pool(name="psmm", bufs=4, space="PSUM") as pmp,
            tc.tile_pool(name="psst", bufs=1, space="PSUM") as pst,
            tc.tile_pool(name="psbc", bufs=1, space="PSUM") as pbc,
        ):
            ones16 = const.tile([P, 1], F16)
            nc.vector.memset(ones16, 1.0)
            onesr = const.tile([1, P], F32)
            nc.vector.memset(onesr, 1.0)
            epsT = const.tile([1, 1], F32)
            nc.vector.memset(epsT, 1e-5)
            ball = const.tile([P, nlin * DT], F16)
            nc.sync.dma_start(out=ball, in_=BRd[:])
            gall = const.tile([P, nln * KT], F32)
            nc.sync.dma_start(out=gall, in_=GRd[:])
            lball = None
            if LBd is not None:
                lball = const.tile([P, nln * KT], F32)
                nc.sync.dma_start(out=lball, in_=LBd[:])

            wt0 = wtp.tile([P, KT, D], F16, name="wt0", tag="wt")
            nc.sync.dma_start(out=wt0[:], in_=WTd[0].rearrange("(k p) d -> p k d", p=P))

            H32 = h32p.tile([P, KT, ns], F32)
            xTr = xT.rearrange("(k p) n -> p k n", p=P)
            cur16 = h16p.tile([P, KT, ns], F16, tag="h16")
            for c in range(nch):
                sl = slice(c * CH, (c + 1) * CH)
                for k in range(KT):
                    nc.sync.dma_start(out=H32[:, k, sl], in_=xTr[:, k, sl])
                for k in range(KT):
                    nc.scalar.copy(out=cur16[:, k, sl], in_=H32[:, k, sl])

            for blk in range(n_blocks):
                has_ln = blk < n_blocks - 1
                # S1 from y3 tiles directly (sum of the carried h0 is ~0 when
                # it is a previous LayerNorm output with trivial gain/bias)
                s1_from_y3 = has_ln and blk > 0 and ln_trivial
                mus = [None] * nch
                mss = [None] * nch
                for li in range(3):
                    i = 3 * blk + li
                    if i == 0:
                        wt = wt0
                    else:
                        wt = wtp.tile([P, KT, D], F16, tag="wt")
                        nc.sync.dma_start(
                            out=wt[:], in_=WTd[i].rearrange("(k p) d -> p k d", p=P)
                        )
                    dst16 = None
                    if li < 2:
                        dst16 = h16p.tile([P, KT, ns], F16, name=f"dst16_{i}", tag="h16")
                    for c in range(nch):
                        sl = slice(c * CH, (c + 1) * CH)
                        s1p = s2p = None
                        if li == 2 and has_ln:
                            s1p = pst.tile([1, CH], F32, name=f"s1_{blk}_{c}", tag="s1")
                            s2p = pst.tile([1, CH], F32, name=f"s2_{blk}_{c}", tag="s2")
                        for d in range(DT):
                            mp = pmp.tile([P, CH], F32, tag="m")
                            for k in range(KT):
                                nc.tensor.matmul(
                                    mp,
                                    lhsT=wt[:, k, d * P : (d + 1) * P],
                                    rhs=cur16[:, k, sl],
                                    start=(k == 0),
                                    stop=(k == KT - 1),
                                )
                            bap = ball[:, i * DT + d : i * DT + d + 1]
                            if li < 2:
                                nc.scalar.activation(
                                    out=dst16[:, d, sl],
                                    in_=mp,
                                    func=AF.Relu,
                                    bias=bap,
                                    scale=1.0,
                                )
                            else:
                                y3 = y3p.tile([P, CH], F16, tag="y3")
                                nc.scalar.activation(
                                    out=y3, in_=mp, func=AF.Identity,
                                    bias=bap, scale=1.0,
                                )
                                nc.vector.tensor_add(
                                    H32[:, d, sl], H32[:, d, sl], y3
                                )
                                if has_ln:
                                    if s1_from_y3:
                                        nc.tensor.matmul(
                                            s1p, lhsT=ones16, rhs=y3,
                                            start=(d == 0), stop=(d == DT - 1),
                                        )
                                    else:
                                        r16 = lnsc.tile([P, CH], F16, tag="r16")
                                        nc.vector.tensor_copy(r16, H32[:, d, sl])
                                        nc.tensor.matmul(
                                            s1p, lhsT=ones16, rhs=r16,
                                            start=(d == 0), stop=(d == DT - 1),
                                        )
                                    rsq = lnsc.tile([P, CH], F16, tag="rsq")
                                    nc.scalar.activation(
                                        out=rsq, in_=H32[:, d, sl], func=AF.Square
                                    )
                                    nc.tensor.matmul(
                                        s2p, lhsT=ones16, rhs=rsq,
                                        start=(d == 0), stop=(d == DT - 1),
                                    )
                        if li == 2 and has_ln:
                            mu_t = mup.tile([1, CH], F32, tag="mu")
                            nc.scalar.mul(out=mu_t, in_=s1p, mul=1.0 / D)
                            ms_t = mup.tile([1, CH], F32, tag="ms")
                            nc.scalar.mul(out=ms_t, in_=s2p, mul=1.0 / D)
                            mus[c] = mu_t
                            mss[c] = ms_t
                    if li < 2:
                        cur16 = dst16

                if has_ln:
                    # ---- LayerNorm over din (partition axis) ----
                    j = blk
                    new16 = h16p.tile([P, KT, ns], F16, name=f"ln16_{j}", tag="h16")
                    for c in range(nch):
                        sl = slice(c * CH, (c + 1) * CH)
                        mu = mus[c]
                        var = rowp.tile([1, CH], F32, tag="var")
                        nc.vector.tensor_mul(var, mu, mu)
                        nc.vector.tensor_sub(var, mss[c], var)
                        # rsqrt via ACT tables: s = exp(-0.5 * ln(var + eps))
                        sd = rowp.tile([1, CH], F32, tag="sd")
                        nc.scalar.activation(
                            out=sd, in_=var, func=AF.Ln, bias=epsT, scale=1.0
                        )
                        srow = rowp.tile([1, CH], F32, tag="srow")
                        nc.scalar.activation(
                            out=srow, in_=sd, func=AF.Exp, bias=0.0, scale=-0.5
                        )
                        bmu = pbc.tile([P, CH], F32, tag="bmu")
                        nc.tensor.matmul(bmu, lhsT=onesr, rhs=mu, start=True, stop=True)
                        bsc = pbc.tile([P, CH], F32, tag="bs")
                        nc.tensor.matmul(bsc, lhsT=onesr, rhs=srow, start=True, stop=True)
                        for k in range(KT):
                            cen = cenp.tile([P, CH], F32, tag="cen")
                            nc.vector.tensor_sub(cen, H32[:, k, sl], bmu)
                            gap = gall[:, j * KT + k : j * KT + k + 1]
                            nc.vector.scalar_tensor_tensor(
                                out=H32[:, k, sl],
                                in0=cen,
                                scalar=gap,
                                in1=bsc,
                                op0=OP.mult,
                                op1=OP.mult,
                            )
                            if lball is not None:
                                lbap = lball[:, j * KT + k : j * KT + k + 1]
                                nc.vector.tensor_scalar_add(
                                    H32[:, k, sl], H32[:, k, sl], lbap
                                )
                            nc.scalar.copy(out=new16[:, k, sl], in_=H32[:, k, sl])
                    cur16 = new16
                else:
                    for k in range(KT):
                        nc.sync.dma_start(
                            out=outT[k * P : (k + 1) * P, :], in_=H32[:, k, :]
                        )
    _split_waits(nc)
    return nc


def _prep_params(W, b, A, Bm, ln_g, ln_b, n_blocks=6):
    nlin = 3 * n_blocks
    nln = max(n_blocks - 1, 1)
    # fold the frozen LoRA path into the main weights: W'' = fp16(W + Bm @ A)
    Wf = (
        W[:nlin].astype(np.float32)
        + np.matmul(Bm[:nlin].astype(np.float32), A[:nlin].astype(np.float32))
    ).astype(np.float16)
    WT = np.ascontiguousarray(Wf.transpose(0, 2, 1))           # [nlin, din, dout] f16
    BR = np.ascontiguousarray(
        b[:nlin].reshape(nlin, DT, P).transpose(2, 0, 1).reshape(P, nlin * DT)
    )
    g = ln_g[:nln] if ln_g.shape[0] >= nln else np.ones((nln, D), np.float32)
    GR = np.ascontiguousarray(
        g.reshape(nln, KT, P).transpose(2, 0, 1).reshape(P, nln * KT)
    ).astype(np.float32)
    lb = ln_b[:nln] if ln_b.shape[0] >= nln else np.zeros((nln, D), np.float32)
    ln_b_nonzero = bool(np.any(lb != 0))
    LB = np.ascontiguousarray(
        lb.reshape(nln, KT, P).transpose(2, 0, 1).reshape(P, nln * KT)
    ).astype(np.float32)
    ln_trivial = bool(np.all(g == 1.0)) and not ln_b_nonzero
    return WT, BR, GR, LB, ln_b_nonzero, ln_trivial


_nc_cache = {}


def run(x, W, b, A, Bm, ln_g, ln_b, n_blocks=6, trace=False, tmpdir=None):
    ns = x.shape[0] // CORES
    WT, BR, GR, LB, ln_b_nonzero, ln_trivial = _prep_params(W, b, A, Bm, ln_g, ln_b, n_blocks)
    key = (ns, n_blocks, ln_b_nonzero, ln_trivial)
    if key not in _nc_cache:
        _nc_cache[key] = build(ns, n_blocks, ln_b_nonzero, ln_trivial)
    nc = _nc_cache[key]

    in_maps = []
    for c in range(CORES):
        m = {
            "xT": np.ascontiguousarray(x[c * ns : (c + 1) * ns, :].T),
            "WT": WT, "BR": BR, "GR": GR,
        }
        if ln_b_nonzero:
            m["LB"] = LB
        in_maps.append(m)

    res = run_bass_kernel_spmd(
        nc, in_maps, list(range(CORES)), trace=trace, tmpdir=tmpdir
    )
    out = np.empty((x.shape[0], D), np.float32)
    for c in range(CORES):
        out[c * ns : (c + 1) * ns, :] = res.results[c]["outT"].T
    return out, res


def kernel(x, W, b, A, Bm, ln_g, ln_b):
    out, _ = run(
        np.asarray(x), np.asarray(W), np.asarray(b), np.asarray(A),
        np.asarray(Bm), np.asarray(ln_g), np.asarray(ln_b),
    )
    return out


# revision 28
# speedup vs baseline: 1.1484x; 1.0230x over previous
"""Trainium2 Bass kernel for nn_LoraBigNet (18x LoRALinear MLP, 6 residual
blocks with inter-block LayerNorm).

Strategy: data-parallel over the batch dim (16384 rows -> 2048 rows/core on 8
cores), parameters replicated. The frozen LoRA low-rank path is folded into
the main weights on the host (W'' = fp16(W + Bm @ A)) — exact same function,
~25% fewer PE cycles on chip. Activations live on-chip in transposed layout
h.T [din(partitions) x n(free)] so every matmul contracts din on the partition
axis with zero on-chip transposes; weights are pre-transposed on the host.
PSUM evacuation applies bias (+ ReLU) on the Scalar engine, rounding to fp16
exactly like the fp16 reference path. LayerNorm reduces over the partition
axis with ones-vector matmuls (for blocks past the first, the sum comes free
from the fp16 residual-branch tiles since the previous LN output has zero
column-sums) and broadcasts mean/rstd back over partitions with K=1 matmuls.
"""

import numpy as np

import concourse.bass as bass
import concourse.mybir as mybir
from concourse.tile import TileContext
from concourse.bass_utils import run_bass_kernel_spmd

F16 = mybir.dt.float16
F32 = mybir.dt.float32
AF = mybir.ActivationFunctionType
OP = mybir.AluOpType

N, D, R, NLIN = 16384, 1024, 32, 18
CORES = 8
NS = N // CORES          # rows per core
KT = D // 128            # contraction tiles
DT = D // 128            # output tiles
CH = 512                 # matmul moving free-dim chunk
P = 128


def _split_waits(nc, maxw=1):
    """This walrus build rejects more than one sync-wait per instruction.
    Hoist extra waits onto preceding same-engine nops — the issuing sequencer
    executes them in order, so the semantics are identical."""
    ctr = 0
    for f in nc.m.functions:
        for bb in f.blocks:
            insts = list(bb.instructions)
            out = []
            changed = False
            for inst in insts:
                si = getattr(inst, "sync_info", None)
                waits = list(si.on_wait) if si and si.on_wait else []
                if len(waits) > maxw:
                    changed = True
                    for w in waits[:-maxw]:
                        nop = mybir.InstNoOp(
                            name=f"wsplit_{ctr}", ins=[], outs=[],
                            engine=inst.engine,
                        )
                        ctr += 1
                        nop.sync_info = mybir.SyncInfo(on_wait=[w], on_update=[])
                        nc.register_instruction(nop, overwrite=True)
                        out.append(nop)
                    inst.sync_info = mybir.SyncInfo(
                        on_wait=waits[-maxw:], on_update=list(si.on_update)
                    )
                out.append(inst)
            if changed:
                bb.instructions = out


def build(ns=NS, n_blocks=6, ln_b_nonzero=False, ln_trivial=True):
    """Build the single-core SPMD Bass program."""
    assert ns % CH == 0
    nlin = 3 * n_blocks
    nln = max(n_blocks - 1, 1)
    nch = ns // CH

    nc = bass.Bass()
    xT = nc.declare_dram_parameter("xT", [D, ns], F32, False)
    WTd = nc.declare_dram_parameter("WT", [nlin, D, D], F16, False)
    BRd = nc.declare_dram_parameter("BR", [P, nlin * DT], F16, False)
    GRd = nc.declare_dram_parameter("GR", [P, nln * KT], F32, False)
    LBd = None
    if ln_b_nonzero:
        LBd = nc.declare_dram_parameter("LB", [P, nln * KT], F32, False)
    outT = nc.declare_dram_parameter("outT", [D, ns], F32, True)

    with TileContext(nc) as tc:
        with (
            tc.tile_pool(name="const", bufs=1) as const,
            tc.tile_pool(name="h32p", bufs=1) as h32p,
            tc.tile_pool(name="h16p", bufs=2) as h16p,
            tc.tile_pool(name="wtp", bufs=2) as wtp,
            tc.tile_pool(name="y3p", bufs=4) as y3p,
            tc.tile_pool(name="lnsc", bufs=2) as lnsc,
            tc.tile_pool(name="rowp", bufs=2) as rowp,
            tc.tile_pool(name="cenp", bufs=2) as cenp,
            tc.tile_pool(name="mup", bufs=4) as mup,
            tc.tile_pool(name="psmm", bufs=4, space="PSUM") as pmp,
            tc.tile_pool(name="psst", bufs=1, space="PSUM") as pst,
            tc.tile_pool(name="psbc", bufs=1, space="PSUM") as pbc,
        ):
            ones16 = const.tile([P, 1], F16)
            nc.vector.memset(ones16, 1.0)
            onesr_f = const.tile([1, P], F32)
            nc.vector.memset(onesr_f, 1.0)
            onesr = const.tile([1, P], mybir.dt.float32r)
            nc.scalar.copy(out=onesr, in_=onesr_f)
            epsT = const.tile([1, 1], F32)
            nc.vector.memset(epsT, 1e-5)
            ball = const.tile([P, nlin * DT], F16)
            nc.sync.dma_start(out=ball, in_=BRd[:])
            gall = const.tile([P, nln * KT], F32)
            nc.sync.dma_start(out=gall, in_=GRd[:])
            lball = None
            if LBd is not None:
                lball = const.tile([P, nln * KT], F32)
                nc.sync.dma_start(out=lball, in_=LBd[:])

            wt0 = wtp.tile([P, KT, D], F16, name="wt0", tag="wt")
            nc.sync.dma_start(out=wt0[:], in_=WTd[0].rearrange("(k p) d -> p k d", p=P))

            H32 = h32p.tile([P, KT, ns], F32)
            xTr = xT.rearrange("(k p) n -> p k n", p=P)
            cur16 = h16p.tile([P, KT, ns], F16, tag="h16")
            for c in range(nch):
                sl = slice(c * CH, (c + 1) * CH)
                for k in range(KT):
                    nc.sync.dma_start(out=H32[:, k, sl], in_=xTr[:, k, sl])
                for k in range(KT):
                    nc.scalar.copy(out=cur16[:, k, sl], in_=H32[:, k, sl])

            for blk in range(n_blocks):
                has_ln = blk < n_blocks - 1
                # S1 from y3 tiles directly (sum of the carried h0 is ~0 when
                # it is a previous LayerNorm output with trivial gain/bias)
                s1_from_y3 = has_ln and blk > 0 and ln_trivial
                mus = [None] * nch
                mss = [None] * nch
                for li in range(3):
                    i = 3 * blk + li
                    if i == 0:
                        wt = wt0
                    else:
                        wt = wtp.tile([P, KT, D], F16, tag="wt")
                        nc.sync.dma_start(
                            out=wt[:], in_=WTd[i].rearrange("(k p) d -> p k d", p=P)
                        )
                    dst16 = None
                    if li < 2:
                        dst16 = h16p.tile([P, KT, ns], F16, name=f"dst16_{i}", tag="h16")
                    for c in range(nch):
                        sl = slice(c * CH, (c + 1) * CH)
                        s1p = s2p = None
                        if li == 2 and has_ln:
                            s1p = pst.tile([1, CH], F32, name=f"s1_{blk}_{c}", tag="s1")
                            s2p = pst.tile([1, CH], F32, name=f"s2_{blk}_{c}", tag="s2")
                        for d in range(DT):
                            mp = pmp.tile([P, CH], F32, tag="m")
                            for k in range(KT):
                                nc.tensor.matmul(
                                    mp,
                                    lhsT=wt[:, k, d * P : (d + 1) * P],
                                    rhs=cur16[:, k, sl],
                                    start=(k == 0),
                                    stop=(k == KT - 1),
                                )
                            bap = ball[:, i * DT + d : i * DT + d + 1]
                            if li < 2:
                                nc.scalar.activation(
                                    out=dst16[:, d, sl],
                                    in_=mp,
                                    func=AF.Relu,
                                    bias=bap,
                                    scale=1.0,
                                )
                            else:
                                y3 = y3p.tile([P, CH], F16, tag="y3")
                                nc.scalar.activation(
                                    out=y3, in_=mp, func=AF.Identity,
                                    bias=bap, scale=1.0,
                                )
                                nc.vector.tensor_add(
                                    H32[:, d, sl], H32[:, d, sl], y3
                                )
                                if not has_ln:
                                    nc.sync.dma_start(
                                        out=outT[d * P : (d + 1) * P, sl],
                                        in_=H32[:, d, sl],
                                    )
                                if has_ln:
                                    if s1_from_y3:
                                        nc.tensor.matmul(
                                            s1p, lhsT=ones16, rhs=y3,
                                            start=(d == 0), stop=(d == DT - 1),
                                        )
                                    else:
                                        r16 = lnsc.tile([P, CH], F16, tag="r16")
                                        nc.vector.tensor_copy(r16, H32[:, d, sl])
                                        nc.tensor.matmul(
                                            s1p, lhsT=ones16, rhs=r16,
                                            start=(d == 0), stop=(d == DT - 1),
                                        )
                                    rsq = lnsc.tile([P, CH], F16, tag="rsq")
                                    nc.scalar.activation(
                                        out=rsq, in_=H32[:, d, sl], func=AF.Square
                                    )
                                    nc.tensor.matmul(
                                        s2p, lhsT=ones16, rhs=rsq,
                                        start=(d == 0), stop=(d == DT - 1),
                                    )
                        if li == 2 and has_ln:
                            mu_t = mup.tile([1, CH], mybir.dt.float32r, tag="mu")
                            nc.scalar.mul(out=mu_t, in_=s1p, mul=1.0 / D)
                            ms_t = mup.tile([1, CH], F32, tag="ms")
                            nc.scalar.mul(out=ms_t, in_=s2p, mul=1.0 / D)
                            mus[c] = mu_t
                            mss[c] = ms_t
                    if li < 2:
                        cur16 = dst16

                if has_ln:
                    # ---- LayerNorm over din (partition axis) ----
                    j = blk
                    new16 = h16p.tile([P, KT, ns], F16, name=f"ln16_{j}", tag="h16")
                    for c in range(nch):
                        sl = slice(c * CH, (c + 1) * CH)
                        mu = mus[c]
                        var = rowp.tile([1, CH], F32, tag="var")
                        nc.vector.tensor_mul(var, mu, mu)
                        nc.vector.tensor_sub(var, mss[c], var)
                        # rsqrt via ACT tables: s = exp(-0.5 * ln(var + eps))
                        sd = rowp.tile([1, CH], F32, tag="sd")
                        nc.scalar.activation(
                            out=sd, in_=var, func=AF.Ln, bias=epsT, scale=1.0
                        )
                        srow = rowp.tile([1, CH], mybir.dt.float32r, tag="srow")
                        nc.scalar.activation(
                            out=srow, in_=sd, func=AF.Exp, bias=0.0, scale=-0.5
                        )
                        bmu = pbc.tile([P, CH], F32, tag="bmu")
                        nc.tensor.matmul(bmu, lhsT=onesr, rhs=mu, start=True, stop=True)
                        bsc = pbc.tile([P, CH], F32, tag="bs")
                        nc.tensor.matmul(bsc, lhsT=onesr, rhs=srow, start=True, stop=True)
                        for k in range(KT):
                            cen = cenp.tile([P, CH], F32, tag="cen")
                            nc.vector.tensor_sub(cen, H32[:, k, sl], bmu)
                            gap = gall[:, j * KT + k : j * KT + k + 1]
                            nc.vector.scalar_tensor_tensor(
                                out=H32[:, k, sl],
                                in0=cen,
                                scalar=gap,
                                in1=bsc,
                                op0=OP.mult,
                                op1=OP.mult,
                            )
                            if lball is not None:
                                lbap = lball[:, j * KT + k : j * KT + k + 1]
                                nc.vector.tensor_scalar_add(
                                    H32[:, k, sl], H32[:, k, sl], lbap
                                )
                            nc.scalar.copy(out=new16[:, k, sl], in_=H32[:, k, sl])
                    cur16 = new16
    _split_waits(nc)
    return nc


def _prep_params(W, b, A, Bm, ln_g, ln_b, n_blocks=6):
    nlin = 3 * n_blocks
    nln = max(n_blocks - 1, 1)
    # fold the frozen LoRA path into the main weights: W'' = fp16(W + Bm @ A)
    Wf = (
        W[:nlin].astype(np.float32)
        + np.matmul(Bm[:nlin].astype(np.float32), A[:nlin].astype(np.float32))
    ).astype(np.float16)
    WT = np.ascontiguousarray(Wf.transpose(0, 2, 1))           # [nlin, din, dout] f16
    BR = np.ascontiguousarray(
        b[:nlin].reshape(nlin, DT, P).transpose(2, 0, 1).reshape(P, nlin * DT)
    )
    g = ln_g[:nln] if ln_g.shape[0] >= nln else np.ones((nln, D), np.float32)
    GR = np.ascontiguousarray(
        g.reshape(nln, KT, P).transpose(2, 0, 1).reshape(P, nln * KT)
    ).astype(np.float32)
    lb = ln_b[:nln] if ln_b.shape[0] >= nln else np.zeros((nln, D), np.float32)
    ln_b_nonzero = bool(np.any(lb != 0))
    LB = np.ascontiguousarray(
        lb.reshape(nln, KT, P).transpose(2, 0, 1).reshape(P, nln * KT)
    ).astype(np.float32)
    ln_trivial = bool(np.all(g == 1.0)) and not ln_b_nonzero
    return WT, BR, GR, LB, ln_b_nonzero, ln_trivial


_nc_cache = {}


def run(x, W, b, A, Bm, ln_g, ln_b, n_blocks=6, trace=False, tmpdir=None):
    ns = x.shape[0] // CORES
    WT, BR, GR, LB, ln_b_nonzero, ln_trivial = _prep_params(W, b, A, Bm, ln_g, ln_b, n_blocks)
    key = (ns, n_blocks, ln_b_nonzero, ln_trivial)
    if key not in _nc_cache:
        _nc_cache[key] = build(ns, n_blocks, ln_b_nonzero, ln_trivial)
    nc = _nc_cache[key]

    in_maps = []
    for c in range(CORES):
        m = {
            "xT": np.ascontiguousarray(x[c * ns : (c + 1) * ns, :].T),
            "WT": WT, "BR": BR, "GR": GR,
        }
        if ln_b_nonzero:
            m["LB"] = LB
        in_maps.append(m)

    res = run_bass_kernel_spmd(
        nc, in_maps, list(range(CORES)), trace=trace, tmpdir=tmpdir
    )
    out = np.empty((x.shape[0], D), np.float32)
    for c in range(CORES):
        out[c * ns : (c + 1) * ns, :] = res.results[c]["outT"].T
    return out, res


def kernel(x, W, b, A, Bm, ln_g, ln_b):
    out, _ = run(
        np.asarray(x), np.asarray(W), np.asarray(b), np.asarray(A),
        np.asarray(Bm), np.asarray(ln_g), np.asarray(ln_b),
    )
    return out


# revision 29
# speedup vs baseline: 1.1510x; 1.0023x over previous
"""Trainium2 Bass kernel for nn_LoraBigNet (18x LoRALinear MLP, 6 residual
blocks with inter-block LayerNorm).

Strategy: data-parallel over the batch dim (16384 rows -> 2048 rows/core on 8
cores), parameters replicated. The frozen LoRA low-rank path is folded into
the main weights on the host (W'' = fp16(W + Bm @ A)) — exact same function,
~25% fewer PE cycles on chip. Activations live on-chip in transposed layout
h.T [din(partitions) x n(free)] so every matmul contracts din on the partition
axis with zero on-chip transposes; weights are pre-transposed on the host.
PSUM evacuation applies bias (+ ReLU) on the Scalar engine, rounding to fp16
exactly like the fp16 reference path. LayerNorm reduces over the partition
axis with ones-vector matmuls (for blocks past the first, the sum comes free
from the fp16 residual-branch tiles since the previous LN output has zero
column-sums) and broadcasts mean/rstd back over partitions with K=1 matmuls.
"""

import numpy as np

import concourse.bass as bass
import concourse.mybir as mybir
from concourse.tile import TileContext
from concourse.bass_utils import run_bass_kernel_spmd

F16 = mybir.dt.float16
F32 = mybir.dt.float32
AF = mybir.ActivationFunctionType
OP = mybir.AluOpType

N, D, R, NLIN = 16384, 1024, 32, 18
CORES = 8
NS = N // CORES          # rows per core
KT = D // 128            # contraction tiles
DT = D // 128            # output tiles
CH = 512                 # matmul moving free-dim chunk
P = 128


def _split_waits(nc, maxw=1):
    """This walrus build rejects more than one sync-wait per instruction.
    Hoist extra waits onto preceding same-engine nops — the issuing sequencer
    executes them in order, so the semantics are identical."""
    ctr = 0
    for f in nc.m.functions:
        for bb in f.blocks:
            insts = list(bb.instructions)
            out = []
            changed = False
            for inst in insts:
                si = getattr(inst, "sync_info", None)
                waits = list(si.on_wait) if si and si.on_wait else []
                if len(waits) > maxw:
                    changed = True
                    for w in waits[:-maxw]:
                        nop = mybir.InstNoOp(
                            name=f"wsplit_{ctr}", ins=[], outs=[],
                            engine=inst.engine,
                        )
                        ctr += 1
                        nop.sync_info = mybir.SyncInfo(on_wait=[w], on_update=[])
                        nc.register_instruction(nop, overwrite=True)
                        out.append(nop)
                    inst.sync_info = mybir.SyncInfo(
                        on_wait=waits[-maxw:], on_update=list(si.on_update)
                    )
                out.append(inst)
            if changed:
                bb.instructions = out


def build(ns=NS, n_blocks=6, ln_b_nonzero=False, ln_trivial=True):
    """Build the single-core SPMD Bass program."""
    assert ns % CH == 0
    nlin = 3 * n_blocks
    nln = max(n_blocks - 1, 1)
    nch = ns // CH

    nc = bass.Bass()
    xT = nc.declare_dram_parameter("xT", [D, ns], F32, False)
    WTd = nc.declare_dram_parameter("WT", [nlin, D, D], F16, False)
    BRd = nc.declare_dram_parameter("BR", [P, nlin * DT], F16, False)
    GRd = nc.declare_dram_parameter("GR", [P, nln * KT], F32, False)
    LBd = None
    if ln_b_nonzero:
        LBd = nc.declare_dram_parameter("LB", [P, nln * KT], F32, False)
    outT = nc.declare_dram_parameter("outT", [D, ns], F32, True)

    with TileContext(nc) as tc:
        with (
            tc.tile_pool(name="const", bufs=1) as const,
            tc.tile_pool(name="h32p", bufs=1) as h32p,
            tc.tile_pool(name="h16p", bufs=2) as h16p,
            tc.tile_pool(name="wtp", bufs=2) as wtp,
            tc.tile_pool(name="y3p", bufs=4) as y3p,
            tc.tile_pool(name="lnsc", bufs=2) as lnsc,
            tc.tile_pool(name="rowp", bufs=2) as rowp,
            tc.tile_pool(name="cenp", bufs=2) as cenp,
            tc.tile_pool(name="mup", bufs=4) as mup,
            tc.tile_pool(name="psmm", bufs=4, space="PSUM") as pmp,
            tc.tile_pool(name="psst", bufs=1, space="PSUM") as pst,
            tc.tile_pool(name="psbc", bufs=1, space="PSUM") as pbc,
        ):
            ones16 = const.tile([P, 1], F16)
            nc.vector.memset(ones16, 1.0)
            onesr_f = const.tile([1, P], F32)
            nc.vector.memset(onesr_f, 1.0)
            onesr = const.tile([1, P], mybir.dt.float32r)
            nc.scalar.copy(out=onesr, in_=onesr_f)
            epsT = const.tile([1, 1], F32)
            nc.vector.memset(epsT, 1e-5)
            ball = const.tile([P, nlin * DT], F16)
            nc.sync.dma_start(out=ball, in_=BRd[:])
            gall = const.tile([P, nln * KT], F32)
            nc.sync.dma_start(out=gall, in_=GRd[:])
            lball = None
            if LBd is not None:
                lball = const.tile([P, nln * KT], F32)
                nc.sync.dma_start(out=lball, in_=LBd[:])

            wt0 = wtp.tile([P, KT, D], F16, name="wt0", tag="wt")
            WT0r = WTd[0].rearrange("(k p) d -> p k d", p=P)
            for k in range(KT):
                nc.sync.dma_start(out=wt0[:, k, :], in_=WT0r[:, k, :])

            H32 = h32p.tile([P, KT, ns], F32)
            xTr = xT.rearrange("(k p) n -> p k n", p=P)
            cur16 = h16p.tile([P, KT, ns], F16, tag="h16")
            for c in range(nch):
                sl = slice(c * CH, (c + 1) * CH)
                for k in range(KT):
                    nc.sync.dma_start(out=H32[:, k, sl], in_=xTr[:, k, sl])
                for k in range(KT):
                    nc.scalar.copy(out=cur16[:, k, sl], in_=H32[:, k, sl])

            for blk in range(n_blocks):
                has_ln = blk < n_blocks - 1
                # S1 from y3 tiles directly (sum of the carried h0 is ~0 when
                # it is a previous LayerNorm output with trivial gain/bias)
                s1_from_y3 = has_ln and blk > 0 and ln_trivial
                mus = [None] * nch
                mss = [None] * nch
                for li in range(3):
                    i = 3 * blk + li
                    if i == 0:
                        wt = wt0
                    else:
                        wt = wtp.tile([P, KT, D], F16, tag="wt")
                        nc.sync.dma_start(
                            out=wt[:], in_=WTd[i].rearrange("(k p) d -> p k d", p=P)
                        )
                    dst16 = None
                    if li < 2:
                        dst16 = h16p.tile([P, KT, ns], F16, name=f"dst16_{i}", tag="h16")
                    for c in range(nch):
                        sl = slice(c * CH, (c + 1) * CH)
                        s1p = s2p = None
                        if li == 2 and has_ln:
                            s1p = pst.tile([1, CH], F32, name=f"s1_{blk}_{c}", tag="s1")
                            s2p = pst.tile([1, CH], F32, name=f"s2_{blk}_{c}", tag="s2")
                        for d in range(DT):
                            mp = pmp.tile([P, CH], F32, tag="m")
                            for k in range(KT):
                                nc.tensor.matmul(
                                    mp,
                                    lhsT=wt[:, k, d * P : (d + 1) * P],
                                    rhs=cur16[:, k, sl],
                                    start=(k == 0),
                                    stop=(k == KT - 1),
                                )
                            bap = ball[:, i * DT + d : i * DT + d + 1]
                            if li < 2:
                                nc.scalar.activation(
                                    out=dst16[:, d, sl],
                                    in_=mp,
                                    func=AF.Relu,
                                    bias=bap,
                                    scale=1.0,
                                )
                            else:
                                y3 = y3p.tile([P, CH], F16, tag="y3")
                                nc.scalar.activation(
                                    out=y3, in_=mp, func=AF.Identity,
                                    bias=bap, scale=1.0,
                                )
                                nc.vector.tensor_add(
                                    H32[:, d, sl], H32[:, d, sl], y3
                                )
                                if not has_ln:
                                    nc.sync.dma_start(
                                        out=outT[d * P : (d + 1) * P, sl],
                                        in_=H32[:, d, sl],
                                    )
                                if has_ln:
                                    if s1_from_y3:
                                        nc.tensor.matmul(
                                            s1p, lhsT=ones16, rhs=y3,
                                            start=(d == 0), stop=(d == DT - 1),
                                        )
                                    else:
                                        r16 = lnsc.tile([P, CH], F16, tag="r16")
                                        nc.vector.tensor_copy(r16, H32[:, d, sl])
                                        nc.tensor.matmul(
                                            s1p, lhsT=ones16, rhs=r16,
                                            start=(d == 0), stop=(d == DT - 1),
                                        )
                                    rsq = lnsc.tile([P, CH], F16, tag="rsq")
                                    nc.scalar.activation(
                                        out=rsq, in_=H32[:, d, sl], func=AF.Square
                                    )
                                    nc.tensor.matmul(
                                        s2p, lhsT=ones16, rhs=rsq,
                                        start=(d == 0), stop=(d == DT - 1),
                                    )
                        if li == 2 and has_ln:
                            mu_t = mup.tile([1, CH], mybir.dt.float32r, tag="mu")
                            nc.scalar.mul(out=mu_t, in_=s1p, mul=1.0 / D)
                            ms_t = mup.tile([1, CH], F32, tag="ms")
                            nc.scalar.mul(out=ms_t, in_=s2p, mul=1.0 / D)
                            mus[c] = mu_t
                            mss[c] = ms_t
                    if li < 2:
                        cur16 = dst16

                if has_ln:
                    # ---- LayerNorm over din (partition axis) ----
                    j = blk
                    new16 = h16p.tile([P, KT, ns], F16, name=f"ln16_{j}", tag="h16")
                    for c in range(nch):
                        sl = slice(c * CH, (c + 1) * CH)
                        mu = mus[c]
                        var = rowp.tile([1, CH], F32, tag="var")
                        nc.vector.tensor_mul(var, mu, mu)
                        nc.vector.tensor_sub(var, mss[c], var)
                        # rsqrt via ACT tables: s = exp(-0.5 * ln(var + eps))
                        sd = rowp.tile([1, CH], F32, tag="sd")
                        nc.scalar.activation(
                            out=sd, in_=var, func=AF.Ln, bias=epsT, scale=1.0
                        )
                        srow = rowp.tile([1, CH], mybir.dt.float32r, tag="srow")
                        nc.scalar.activation(
                            out=srow, in_=sd, func=AF.Exp, bias=0.0, scale=-0.5
                        )
                        bmu = pbc.tile([P, CH], F32, tag="bmu")
                        nc.tensor.matmul(bmu, lhsT=onesr, rhs=mu, start=True, stop=True)
                        bsc = pbc.tile([P, CH], F32, tag="bs")
                        nc.tensor.matmul(bsc, lhsT=onesr, rhs=srow, start=True, stop=True)
                        for k in range(KT):
                            cen = cenp.tile([P, CH], F32, tag="cen")
                            nc.vector.tensor_sub(cen, H32[:, k, sl], bmu)
                            gap = gall[:, j * KT + k : j * KT + k + 1]
                            nc.vector.scalar_tensor_tensor(
                                out=H32[:, k, sl],
                                in0=cen,
                                scalar=gap,
                                in1=bsc,
                                op0=OP.mult,
                                op1=OP.mult,
                            )
                            if lball is not None:
                                lbap = lball[:, j * KT + k : j * KT + k + 1]
                                nc.vector.tensor_scalar_add(
                                    H32[:, k, sl], H32[:, k, sl], lbap
                                )
                            nc.scalar.copy(out=new16[:, k, sl], in_=H32[:, k, sl])
                    cur16 = new16
    _split_waits(nc)
    return nc


def _prep_params(W, b, A, Bm, ln_g, ln_b, n_blocks=6):
    nlin = 3 * n_blocks
    nln = max(n_blocks - 1, 1)
    # fold the frozen LoRA path into the main weights: W'' = fp16(W + Bm @ A)
    Wf = (
        W[:nlin].astype(np.float32)
        + np.matmul(Bm[:nlin].astype(np.float32), A[:nlin].astype(np.float32))
    ).astype(np.float16)
    WT = np.ascontiguousarray(Wf.transpose(0, 2, 1))           # [nlin, din, dout] f16
    BR = np.ascontiguousarray(
        b[:nlin].reshape(nlin, DT, P).transpose(2, 0, 1).reshape(P, nlin * DT)
    )
    g = ln_g[:nln] if ln_g.shape[0] >= nln else np.ones((nln, D), np.float32)
    GR = np.ascontiguousarray(
        g.reshape(nln, KT, P).transpose(2, 0, 1).reshape(P, nln * KT)
    ).astype(np.float32)
    lb = ln_b[:nln] if ln_b.shape[0] >= nln else np.zeros((nln, D), np.float32)
    ln_b_nonzero = bool(np.any(lb != 0))
    LB = np.ascontiguousarray(
        lb.reshape(nln, KT, P).transpose(2, 0, 1).reshape(P, nln * KT)
    ).astype(np.float32)
    ln_trivial = bool(np.all(g == 1.0)) and not ln_b_nonzero
    return WT, BR, GR, LB, ln_b_nonzero, ln_trivial


_nc_cache = {}


def run(x, W, b, A, Bm, ln_g, ln_b, n_blocks=6, trace=False, tmpdir=None):
    ns = x.shape[0] // CORES
    WT, BR, GR, LB, ln_b_nonzero, ln_trivial = _prep_params(W, b, A, Bm, ln_g, ln_b, n_blocks)
    key = (ns, n_blocks, ln_b_nonzero, ln_trivial)
    if key not in _nc_cache:
        _nc_cache[key] = build(ns, n_blocks, ln_b_nonzero, ln_trivial)
    nc = _nc_cache[key]

    in_maps = []
    for c in range(CORES):
        m = {
            "xT": np.ascontiguousarray(x[c * ns : (c + 1) * ns, :].T),
            "WT": WT, "BR": BR, "GR": GR,
        }
        if ln_b_nonzero:
            m["LB"] = LB
        in_maps.append(m)

    res = run_bass_kernel_spmd(
        nc, in_maps, list(range(CORES)), trace=trace, tmpdir=tmpdir
    )
    out = np.empty((x.shape[0], D), np.float32)
    for c in range(CORES):
        out[c * ns : (c + 1) * ns, :] = res.results[c]["outT"].T
    return out, res


def kernel(x, W, b, A, Bm, ln_g, ln_b):
    out, _ = run(
        np.asarray(x), np.asarray(W), np.asarray(b), np.asarray(A),
        np.asarray(Bm), np.asarray(ln_g), np.asarray(ln_b),
    )
    return out
